# revision 21
# baseline (speedup 1.0000x reference)
"""CDMF segment-reduce kernel for 8 Trainium2 NeuronCores.

Strategy (fast path, alpha=beta=gamma=1)
----------------------------------------
Host (index-only prep + gathers):
  * stable-sort rows by user id; cut the 100k rows into 8 shards at user
    boundaries (expert-style user sharding) so each core owns a disjoint
    user range -> no cross-core reduction at all.
  * greedy-pack tiles (128 rows) into GROUPS such that the number of
    distinct users first seen in a group stays <= 128: each group owns ONE
    PSUM accumulator bank; a user spans at most 2 consecutive groups.
  * within each group, rows of the user continuing from the previous group
    are pinned first (tile 0), remaining rows sorted by valid-count so each
    tile's max sequence length is small.
  * per-row compaction: only the masked-in (valid) steps of R are shipped,
    compacted to the front; each tile t ships [128, SP[t], 64] where SP[t]
    is the tile's max valid-count. This roughly halves both HBM traffic
    and vector work.
  * R/w/mask are rounded to bf16 (2x DVE, half DMA); the weighted-sum /
    normalize path stays f32.

Device (one SPMD program, 8 cores):
  * per tile: y = R*w (DVE bf16), fold-tree reduce over d (DVE bf16, last
    folds f32), wt = sum_s maskc * max(z, tau) (GPSIMD stt with f32 accum)
  * X = [wt*q | wt] (ACT); one-hot matmul scatters X into the group's PSUM
    bank (PE f32); bank flushed to SBUF once per group (ACT)
  * gather: per tile one-hot^T matmul pulls [num|den] per row from its
    user's bank (+ prev-group bank for boundary rows); r = sum_e num*q via
    fused tensor_tensor_reduce (DVE); one reciprocal+mul for all tiles.
"""

import numpy as np
import ml_dtypes

import concourse.bass as bass
import concourse.tile as tile
from concourse import bacc, mybir
from concourse.bass_utils import run_bass_kernel_spmd

N_CORES = 8
TAU = 0.01
S = 50          # seq_len
D = 64          # n_features
E = 128         # emb_dim
F32 = mybir.dt.float32
BF16 = mybir.dt.bfloat16
LCAP = 12       # max tiles per bank-group

BF = ml_dtypes.bfloat16


def _round_bf16(x):
    """Round f32 array to bf16 (round-to-nearest-even), as ml_dtypes array."""
    x = np.ascontiguousarray(x, np.float32)
    u = x.view(np.uint32)
    r = ((u + np.uint32(0x7FFF) + ((u >> np.uint32(16)) & np.uint32(1)))
         >> np.uint32(16)).astype(np.uint16)
    return r.view(BF)


# ----------------------------------------------------------------------------
# host-side preprocessing (fast path)
# ----------------------------------------------------------------------------

def _preprocess(users, items, R_ui, mask, w, item_emb, fast=True):
    n = users.shape[0]
    perm = np.argsort(users, kind="stable")
    users_s = users[perm]
    cnt_all = mask.sum(1).astype(np.int64)      # [n] valid counts (>=1)

    # shard cuts at user boundaries
    cuts = [0]
    for c in range(1, N_CORES):
        t = round(c * n / N_CORES)
        while 0 < t < n and users_s[t] == users_s[t - 1]:
            t += 1
        cuts.append(min(t, n))
    cuts.append(n)
    sizes = [cuts[c + 1] - cuts[c] for c in range(N_CORES)]
    NT = max(1, int(np.ceil(max(sizes) / 128)))
    NPAD = NT * 128

    # per-core row order (original indices; -1 = padding) in user-sorted order
    orders = np.full((N_CORES, NPAD), -1, np.int64)
    for c in range(N_CORES):
        lo, hi = cuts[c], cuts[c + 1]
        orders[c, : hi - lo] = perm[lo:hi]

    # first-occurrence flags per core (pads: 0)
    firstF = np.zeros((N_CORES, NPAD + 1), np.int64)  # cumulative
    uarr = np.full((N_CORES, NPAD), -1, np.int64)
    for c in range(N_CORES):
        u = np.where(orders[c] >= 0, users[np.maximum(orders[c], 0)], -1)
        uarr[c] = u
        f = np.zeros(NPAD, np.int64)
        real = u >= 0
        f[real] = np.r_[1, (u[real][1:] != u[real][:-1]).astype(np.int64)]
        firstF[c, 1:] = np.cumsum(f)

    # greedy group packing: extend while new-user slots <= 128 on all cores
    GS = []
    t0 = 0
    while t0 < NT:
        L = 1
        while (L < LCAP and t0 + L < NT and
               max(firstF[c, (t0 + L + 1) * 128] - firstF[c, t0 * 128]
                   for c in range(N_CORES)) <= 128):
            L += 1
        GS.append(L)
        t0 += L
    NG = len(GS)
    gstarts = np.r_[0, np.cumsum(GS)][:-1]

    # within-group reorder: boundary-user rows first, then by cnt desc
    for c in range(N_CORES):
        u_orig = uarr[c].copy()   # user ids in the pristine sorted order
        for g in range(NG):
            r0, r1 = gstarts[g] * 128, (gstarts[g] + GS[g]) * 128
            ug = u_orig[r0:r1]
            og = orders[c, r0:r1].copy()
            if r0 > 0 and ug[0] >= 0 and ug[0] == u_orig[r0 - 1]:
                # count of leading rows belonging to the boundary user
                nb = int((ug == ug[0]).cumprod().sum())
            else:
                nb = 0
            assert nb <= 128, f"boundary user has {nb} rows in group"
            rest = og[nb:]
            restc = np.where(rest >= 0, cnt_all[np.maximum(rest, 0)], -1)
            so = np.argsort(-restc, kind="stable")
            orders[c, r0 + nb:r1] = rest[so]
        # refresh user array after reorder
        uarr[c] = np.where(orders[c] >= 0, users[np.maximum(orders[c], 0)], -1)

    # per-tile compacted seq lengths (shared across cores)
    cnt_rows = np.where(orders >= 0, cnt_all[np.maximum(orders, 0)], 0)  # [C, NPAD]
    SP = np.maximum(cnt_rows.reshape(N_CORES, NT, 128).max(-1).max(0), 1)  # [NT]
    SMAX = int(SP.max())
    roffs = np.r_[0, np.cumsum(SP * D)][:-1]
    moffs = np.r_[0, np.cumsum(SP)][:-1]
    CW = int((SP * D).sum())
    CM = int(SP.sum())
    TOTL = int(2 * sum(GS))
    loffs = np.r_[0, np.cumsum([2 * L for L in GS])][:-1]

    plan = {
        "NT": NT, "GS": GS, "SP": [int(x) for x in SP],
        "SMAX": SMAX, "CW": CW, "CM": CM, "TOTL": TOTL, "NG": NG,
        "roffs": [int(x) for x in roffs], "moffs": [int(x) for x in moffs],
        "loffs": [int(x) for x in loffs],
        "gstarts": [int(x) for x in gstarts],
    }

    wrep = _round_bf16(np.broadcast_to(w[None, None, :], (128, SMAX, D)))

    in_maps = []
    metas = []
    for c in range(N_CORES):
        order = orders[c]
        real = order >= 0
        ridx = np.maximum(order, 0)

        # gather + compact R rows
        Rg = R_ui[ridx]                           # [NPAD, S, D]
        mg = mask[ridx] > 0
        vorder = np.argsort(~mg, axis=1, kind="stable")   # valid steps first
        Rc = np.take_along_axis(Rg, vorder[:, :, None], axis=1)
        Rc[~real] = 0.0
        cr = cnt_rows[c]                          # [NPAD]
        # zero out the tail beyond each row's count (compacted invalid steps)
        tailmask = np.arange(S)[None, :] >= cr[:, None]
        Rc[tailmask] = 0.0

        Rp2d = np.zeros((128, CW), np.float32)
        maskc = np.zeros((128, CM), np.float32)
        cntw = np.zeros((128, NT), np.float32)
        for t in range(NT):
            sp = int(SP[t])
            blk = Rc[t * 128:(t + 1) * 128, :sp, :].reshape(128, sp * D)
            Rp2d[:, roffs[t]:roffs[t] + sp * D] = blk
            ct = cr[t * 128:(t + 1) * 128]
            valid = np.arange(sp)[None, :] < ct[:, None]
            # fast path folds the *cnt scale into the mask so one fused op
            # yields wt directly; general path needs the raw 0/1 mask
            maskc[:, moffs[t]:moffs[t] + sp] = (
                valid * ct[:, None] if fast else valid)
            cntw[:, t] = ct

        q = item_emb[items[ridx]].astype(np.float32)
        q[~real] = 0.0
        qw = np.ascontiguousarray(
            q.reshape(NT, 128, E).transpose(1, 0, 2))   # [128, NT, E]

        # one-hot matrices per group
        u = uarr[c]
        ohs = np.zeros((128, TOTL, 128), np.float32)
        bnds = np.zeros((128, max(2 * (NG - 1), 1), 128), np.float32)
        prev_slot = {}
        for g in range(NG):
            r0, r1 = gstarts[g] * 128, (gstarts[g] + GS[g]) * 128
            ug = u[r0:r1]
            slot = {}
            nslot = 0
            lo = loffs[g]
            L = GS[g]
            for k in range(r1 - r0):
                uu = ug[k]
                if uu < 0:
                    continue
                j, p = divmod(k, 128)
                if g > 0 and uu in prev_slot:
                    # boundary row: scatter into previous group's bank
                    assert j == 0, "boundary row outside tile 0"
                    s0 = prev_slot[uu]
                    bnds[p, 2 * (g - 1), s0] = 1.0      # seg closer
                    bnds[s0, 2 * (g - 1) + 1, p] = 1.0  # gather
                else:
                    if uu not in slot:
                        slot[uu] = nslot
                        nslot += 1
                    s0 = slot[uu]
                    ohs[p, lo + j, s0] = 1.0            # seg own
                    ohs[s0, lo + L + j, p] = 1.0        # gather own
            assert nslot <= 128, f"slot overflow {nslot}"
            prev_slot = slot

        in_maps.append({
            "Rp2d": _round_bf16(Rp2d),
            "maskc": _round_bf16(maskc),
            "cntw": cntw,
            "qw": qw,
            "wrep": wrep,
            "ohs": ohs,
            "bnds": bnds,
        })
        metas.append(order)
    return in_maps, metas, plan


# ----------------------------------------------------------------------------
# device program (fast path)
# ----------------------------------------------------------------------------

def build_program(plan, alpha=1.0, beta=1.0, gamma=1.0):
    nc = bacc.Bacc(
        "TRN2", target_bir_lowering=False, debug=False, num_devices=N_CORES
    )
    fast = (alpha == 1.0) and (beta == 1.0) and (gamma == 1.0)
    AF = mybir.ActivationFunctionType
    NT, GS, SP = plan["NT"], plan["GS"], plan["SP"]
    SMAX, CW, CM, TOTL, NG = (plan["SMAX"], plan["CW"], plan["CM"],
                              plan["TOTL"], plan["NG"])
    roffs, moffs, loffs = plan["roffs"], plan["moffs"], plan["loffs"]
    gstarts = plan["gstarts"]

    Rp2d = nc.dram_tensor("Rp2d", [128, CW], BF16, kind="ExternalInput")
    maskc_d = nc.dram_tensor("maskc", [128, CM], BF16, kind="ExternalInput")
    cntw_d = nc.dram_tensor("cntw", [128, NT], F32, kind="ExternalInput")
    qw_d = nc.dram_tensor("qw", [128, NT, E], F32, kind="ExternalInput")
    wrep_d = nc.dram_tensor("wrep", [128, SMAX, D], BF16, kind="ExternalInput")
    ohs_d = nc.dram_tensor("ohs", [128, TOTL, 128], F32, kind="ExternalInput")
    bnds_d = nc.dram_tensor(
        "bnds", [128, max(2 * (NG - 1), 1), 128], F32, kind="ExternalInput")
    r_out = nc.dram_tensor("r_out", [128, NT], F32, kind="ExternalOutput")

    add = mybir.AluOpType.add
    mult = mybir.AluOpType.mult
    mx = mybir.AluOpType.max

    with tile.TileContext(nc) as tc:
        with (
            tc.tile_pool(name="const", bufs=1) as constp,
            tc.tile_pool(name="rpool", bufs=4) as rpool,
            tc.tile_pool(name="ypool", bufs=2) as ypool,
            tc.tile_pool(name="fpool", bufs=2) as fpool,
            tc.tile_pool(name="zpool", bufs=2) as zpool,
            tc.tile_pool(name="qpool", bufs=4) as qpool,
            tc.tile_pool(name="ohpool", bufs=3) as ohpool,
            tc.tile_pool(name="bndpool", bufs=3) as bndpool,
            tc.tile_pool(name="xpool", bufs=4) as xpool,
            tc.tile_pool(name="scpool", bufs=2) as scpool,
            tc.tile_pool(name="small", bufs=4) as small,
            tc.tile_pool(name="bankp", bufs=1) as bankp,
            tc.tile_pool(name="pseg", bufs=2, space="PSUM") as pseg,
            tc.tile_pool(name="pgat", bufs=3, space="PSUM") as pgat,
        ):
            wrep_sb = constp.tile([128, SMAX, D], BF16)
            nc.sync.dma_start(wrep_sb[:], wrep_d[:, :, :])
            if not fast:
                maskc_sb = constp.tile([128, CM], BF16)
                nc.sync.dma_start(maskc_sb[:], maskc_d[:, :])
            bank_sb = bankp.tile([128, NG, E + 1], F32)
            wt_sb = constp.tile([128, NT], F32)
            den_sb = constp.tile([128, NT], F32)
            r_sb = constp.tile([128, NT], F32)
            cnt_sb = constp.tile([128, NT], F32)
            nc.sync.dma_start(cnt_sb[:], cntw_d[:, :])
            if fast:
                # c2 = tau * cnt^2 (bias for the wt combine)
                c2_sb = constp.tile([128, NT], F32)
                nc.vector.scalar_tensor_tensor(
                    c2_sb[:], cnt_sb[:], TAU, cnt_sb[:],
                    op0=mult, op1=mult)
                negtau = constp.tile([128, 1], F32)
                nc.vector.memset(negtau[:], -TAU)

            q_tiles = [None] * NG
            oh_tiles = [None] * NG
            bnd_tiles = [None] * NG
            ps_prev = None

            def emit_gather(g):
                t0, L = gstarts[g], GS[g]
                ohg = oh_tiles[g]
                qg = q_tiles[g]
                for j in range(L):
                    t = t0 + j
                    gp = pgat.tile([128, E + 1], F32, name="gp")
                    if g > 0 and j == 0:
                        nc.tensor.matmul(
                            gp[:], bnd_tiles[g][:, 1, :], bank_sb[:, g - 1, :],
                            start=True, stop=False)
                        nc.tensor.matmul(
                            gp[:], ohg[:, L + j, :], bank_sb[:, g, :],
                            start=False, stop=True)
                    else:
                        nc.tensor.matmul(
                            gp[:], ohg[:, L + j, :], bank_sb[:, g, :],
                            start=True, stop=True)
                    nc.scalar.copy(den_sb[:, t:t + 1], gp[:, E:E + 1])
                    # r_num = sum_e num*q, fused mul+accum on DVE
                    # (tensor_tensor_reduce is a direct-ISA op that the
                    # PJRT execution path cannot run; stt is equivalent here)
                    sc = scpool.tile([128, E], F32, name="sc")
                    nc.vector.scalar_tensor_tensor(
                        sc[:], gp[:, 0:E], 1.0, qg[:, j, :],
                        op0=mult, op1=mult, accum_out=r_sb[:, t:t + 1])

            # ---- phase A: Z -> wt -> X -> segment matmuls into group banks
            for g in range(NG):
                t0, L = gstarts[g], GS[g]
                ohg = ohpool.tile([128, 2 * LCAP, 128], F32)
                nc.scalar.dma_start(
                    ohg[:, 0:2 * L, :], ohs_d[:, loffs[g]:loffs[g] + 2 * L, :])
                oh_tiles[g] = ohg
                if g > 0:
                    bnd = bndpool.tile([128, 2, 128], F32)
                    nc.scalar.dma_start(
                        bnd[:], bnds_d[:, 2 * (g - 1):2 * g, :])
                    bnd_tiles[g] = bnd
                qg = qpool.tile([128, LCAP, E], F32)
                nc.scalar.dma_start(qg[:, 0:L, :], qw_d[:, t0:t0 + L, :])
                q_tiles[g] = qg

                ps = pseg.tile([128, E + 1], F32)
                for j in range(L):
                    t = t0 + j
                    sp = SP[t]
                    rt = rpool.tile([128, SMAX * D], BF16)
                    nc.sync.dma_start(
                        rt[:, 0:sp * D], Rp2d[:, roffs[t]:roffs[t] + sp * D])
                    rv = rt[:, 0:sp * D].rearrange("p (s d) -> p s d", d=D)
                    y = ypool.tile([128, SMAX, D], BF16)
                    nc.vector.tensor_mul(
                        y[:, 0:sp, :], rv, wrep_sb[:, 0:sp, :])
                    f1 = fpool.tile([128, SMAX, 32], BF16, tag="f1")
                    nc.vector.tensor_add(
                        f1[:, 0:sp, :], y[:, 0:sp, 0:32], y[:, 0:sp, 32:64])
                    f2 = fpool.tile([128, SMAX, 16], BF16, tag="f2")
                    nc.vector.tensor_add(
                        f2[:, 0:sp, :], f1[:, 0:sp, 0:16], f1[:, 0:sp, 16:32])
                    f3 = fpool.tile([128, SMAX, 8], BF16, tag="f3")
                    nc.vector.tensor_add(
                        f3[:, 0:sp, :], f2[:, 0:sp, 0:8], f2[:, 0:sp, 8:16])
                    f4 = fpool.tile([128, SMAX, 4], BF16, tag="f4")
                    nc.vector.tensor_add(
                        f4[:, 0:sp, :], f3[:, 0:sp, 0:4], f3[:, 0:sp, 4:8])
                    f5 = fpool.tile([128, SMAX, 2], F32, tag="f5")
                    nc.vector.tensor_add(
                        f5[:, 0:sp, :], f4[:, 0:sp, 0:2], f4[:, 0:sp, 2:4])
                    z = zpool.tile([128, SMAX], F32, tag="z")
                    nc.vector.tensor_add(
                        z[:, 0:sp], f5[:, 0:sp, 0], f5[:, 0:sp, 1])

                    wt_col = wt_sb[:, t:t + 1]
                    if fast:
                        # compacted tail is exactly 0 so relu(z - tau) drops
                        # it: wt = cnt*sum_s max(z,tau) = cnt*A + tau*cnt^2
                        # with A = sum_s relu(z - tau). Runs on ACT only.
                        za = zpool.tile([128, SMAX], F32, tag="za")
                        a_col = small.tile([128, 1], F32, tag="a")
                        nc.scalar.activation(
                            za[:, 0:sp], z[:, 0:sp], AF.Relu, bias=negtau[:],
                            accum_out=a_col)
                        nc.scalar.activation(
                            wt_col, a_col[:], AF.Relu,
                            bias=c2_sb[:, t:t + 1], scale=cnt_sb[:, t:t + 1])
                    else:
                        wp = zpool.tile([128, SMAX], F32, tag="wp")
                        # A = sum_s mask * max(z,tau)^alpha;
                        # wt = exp((gamma/alpha)*(ln A + alpha*beta*ln cnt))
                        za = zpool.tile([128, SMAX], F32, tag="za")
                        nc.vector.tensor_scalar_max(
                            za[:, 0:sp], z[:, 0:sp], TAU)
                        nc.scalar.activation(za[:, 0:sp], za[:, 0:sp], AF.Log)
                        nc.scalar.activation(
                            za[:, 0:sp], za[:, 0:sp], AF.Exp,
                            scale=float(alpha))
                        a_col = small.tile([128, 1], F32, tag="a")
                        nc.vector.scalar_tensor_tensor(
                            wp[:, 0:sp], za[:, 0:sp], 0.0,
                            maskc_sb[:, moffs[t]:moffs[t] + sp],
                            op0=add, op1=mult, accum_out=a_col)
                        la = small.tile([128, 1], F32, tag="la")
                        nc.scalar.activation(la[:], a_col[:], AF.Log)
                        lc = small.tile([128, 1], F32, tag="lc")
                        nc.scalar.activation(
                            lc[:], cnt_sb[:, t:t + 1], AF.Log)
                        nc.vector.scalar_tensor_tensor(
                            la[:], lc[:], float(alpha * beta), la[:],
                            op0=mult, op1=add)
                        nc.scalar.activation(
                            wt_col, la[:], AF.Exp, scale=float(gamma / alpha))

                    xt = xpool.tile([128, E + 1], F32)
                    nc.scalar.mul(xt[:, 0:E], qg[:, j, :], wt_col)
                    nc.scalar.copy(xt[:, E:E + 1], wt_col)

                    last = (g == NG - 1) and (j == L - 1)
                    nc.tensor.matmul(
                        ps[:], ohg[:, j, :], xt[:],
                        start=(j == 0), stop=last)
                    if j == 0 and g > 0:
                        nc.tensor.matmul(
                            ps_prev[:], bnd_tiles[g][:, 0, :], xt[:],
                            start=False, stop=True)
                        nc.scalar.copy(bank_sb[:, g - 1, :], ps_prev[:])
                    if last:
                        nc.scalar.copy(bank_sb[:, g, :], ps[:])
                ps_prev = ps

                # gather phase for group g-1 (its bank just closed); keeps
                # the DVE/PE streams interleaved so pool recycling never
                # stalls the segment pipeline behind a long gather tail
                if g > 0:
                    emit_gather(g - 1)
            emit_gather(NG - 1)

            # pad lanes have den=0 (all-zero one-hot rows); clamp so the
            # reciprocal stays finite and r stays 0 instead of NaN
            rec = constp.tile([128, NT], F32)
            nc.vector.tensor_scalar_max(den_sb[:], den_sb[:], 1e-30)
            nc.vector.reciprocal(rec[:], den_sb[:])
            nc.vector.tensor_mul(r_sb[:], r_sb[:], rec[:])
            nc.sync.dma_start(r_out[:, :], r_sb[:])

    nc.compile()
    return nc


# ----------------------------------------------------------------------------
# entry point
# ----------------------------------------------------------------------------

def kernel(users, items, R_ui, mask, w, item_emb, alpha, beta, gamma,
           _return_extras=False, _trace=False):
    users = np.asarray(users, np.int64)
    items = np.asarray(items, np.int64)
    R_ui = np.asarray(R_ui, np.float32)
    mask_f = np.asarray(mask).astype(np.float32)
    w = np.asarray(w, np.float32)
    item_emb = np.asarray(item_emb, np.float32)
    al = float(np.asarray(alpha).reshape(-1)[0])
    be = float(np.asarray(beta).reshape(-1)[0])
    ga = float(np.asarray(gamma).reshape(-1)[0])

    fast = (al == 1.0) and (be == 1.0) and (ga == 1.0)

    import time as _time

    t0 = _time.perf_counter()
    in_maps, metas, plan = _preprocess(
        users, items, R_ui, mask_f, w, item_emb, fast=fast)
    t1 = _time.perf_counter()
    nc = build_program(plan, al, be, ga)
    t2 = _time.perf_counter()
    res = run_bass_kernel_spmd(
        nc, in_maps, core_ids=list(range(N_CORES)), trace=_trace
    )
    t3 = _time.perf_counter()
    print(
        f"[kernel] preprocess {t1-t0:.1f}s  build+schedule {t2-t1:.1f}s  "
        f"compile+run {t3-t2:.1f}s"
    )

    n = users.shape[0]
    r = np.empty(n, np.float32)
    for c in range(N_CORES):
        order = metas[c]
        flat = np.asarray(res.results[c]["r_out"], np.float32).T.reshape(-1)
        valid = order >= 0
        r[order[valid]] = flat[valid]
    if _return_extras:
        return r, res
    return r


# revision 39
# speedup vs baseline: 1.1741x; 1.1741x over previous
"""CDMF segment-reduce kernel for 8 Trainium2 NeuronCores.

Strategy (fast path, alpha=beta=gamma=1)
----------------------------------------
Host (index-only prep + gathers):
  * stable-sort rows by user id; cut the 100k rows into 8 shards at user
    boundaries (expert-style user sharding) so each core owns a disjoint
    user range -> no cross-core reduction at all.
  * greedy-pack tiles (128 rows) into GROUPS such that the number of
    distinct users first seen in a group stays <= 128: each group owns ONE
    PSUM accumulator bank; a user spans at most 2 consecutive groups.
  * within each group, rows of the user continuing from the previous group
    are pinned first (tile 0), remaining rows sorted by valid-count so each
    tile's max sequence length is small.
  * per-row compaction: only the masked-in (valid) steps of R are shipped,
    compacted to the front; each tile t ships [128, SP[t], 64] where SP[t]
    is the tile's max valid-count. This roughly halves both HBM traffic
    and vector work.
  * R/w/mask are rounded to bf16 (2x DVE, half DMA); the weighted-sum /
    normalize path stays f32.

Device (one SPMD program, 8 cores):
  * per tile: y = R*w (DVE bf16), fold-tree reduce over d (DVE bf16, last
    folds f32), wt = sum_s maskc * max(z, tau) (GPSIMD stt with f32 accum)
  * X = [wt*q | wt] (ACT); one-hot matmul scatters X into the group's PSUM
    bank (PE f32); bank flushed to SBUF once per group (ACT)
  * gather: per tile one-hot^T matmul pulls [num|den] per row from its
    user's bank (+ prev-group bank for boundary rows); r = sum_e num*q via
    fused tensor_tensor_reduce (DVE); one reciprocal+mul for all tiles.
"""

import numpy as np
import ml_dtypes

import concourse.bass as bass
import concourse.tile as tile
from concourse import bacc, mybir
from concourse.bass_utils import run_bass_kernel_spmd

N_CORES = 8
TAU = 0.01
S = 50          # seq_len
D = 64          # n_features
E = 128         # emb_dim
F32 = mybir.dt.float32
BF16 = mybir.dt.bfloat16
LCAP = 12       # max tiles per bank-group

BF = ml_dtypes.bfloat16


def _round_bf16(x):
    """Round f32 array to bf16 (round-to-nearest-even), as ml_dtypes array."""
    x = np.ascontiguousarray(x, np.float32)
    u = x.view(np.uint32)
    r = ((u + np.uint32(0x7FFF) + ((u >> np.uint32(16)) & np.uint32(1)))
         >> np.uint32(16)).astype(np.uint16)
    return r.view(BF)


# ----------------------------------------------------------------------------
# host-side preprocessing (fast path)
# ----------------------------------------------------------------------------

def _preprocess(users, items, R_ui, mask, w, item_emb, fast=True):
    n = users.shape[0]
    perm = np.argsort(users, kind="stable")
    users_s = users[perm]
    cnt_all = mask.sum(1).astype(np.int64)      # [n] valid counts (>=1)

    # shard cuts at user boundaries
    cuts = [0]
    for c in range(1, N_CORES):
        t = round(c * n / N_CORES)
        while 0 < t < n and users_s[t] == users_s[t - 1]:
            t += 1
        cuts.append(min(t, n))
    cuts.append(n)
    sizes = [cuts[c + 1] - cuts[c] for c in range(N_CORES)]
    NT = max(1, int(np.ceil(max(sizes) / 128)))
    NPAD = NT * 128

    # per-core row order (original indices; -1 = padding) in user-sorted order
    orders = np.full((N_CORES, NPAD), -1, np.int64)
    for c in range(N_CORES):
        lo, hi = cuts[c], cuts[c + 1]
        orders[c, : hi - lo] = perm[lo:hi]

    # first-occurrence flags per core (pads: 0)
    firstF = np.zeros((N_CORES, NPAD + 1), np.int64)  # cumulative
    uarr = np.full((N_CORES, NPAD), -1, np.int64)
    for c in range(N_CORES):
        u = np.where(orders[c] >= 0, users[np.maximum(orders[c], 0)], -1)
        uarr[c] = u
        f = np.zeros(NPAD, np.int64)
        real = u >= 0
        f[real] = np.r_[1, (u[real][1:] != u[real][:-1]).astype(np.int64)]
        firstF[c, 1:] = np.cumsum(f)

    # greedy group packing: extend while new-user slots <= 128 on all cores
    GS = []
    t0 = 0
    while t0 < NT:
        L = 1
        while (L < LCAP and t0 + L < NT and
               max(firstF[c, (t0 + L + 1) * 128] - firstF[c, t0 * 128]
                   for c in range(N_CORES)) <= 128):
            L += 1
        GS.append(L)
        t0 += L
    NG = len(GS)
    gstarts = np.r_[0, np.cumsum(GS)][:-1]

    # within-group reorder: boundary-user rows first, then by cnt desc
    for c in range(N_CORES):
        u_orig = uarr[c].copy()   # user ids in the pristine sorted order
        for g in range(NG):
            r0, r1 = gstarts[g] * 128, (gstarts[g] + GS[g]) * 128
            ug = u_orig[r0:r1]
            og = orders[c, r0:r1].copy()
            if r0 > 0 and ug[0] >= 0 and ug[0] == u_orig[r0 - 1]:
                # count of leading rows belonging to the boundary user
                nb = int((ug == ug[0]).cumprod().sum())
            else:
                nb = 0
            assert nb <= 128, f"boundary user has {nb} rows in group"
            rest = og[nb:]
            restc = np.where(rest >= 0, cnt_all[np.maximum(rest, 0)], -1)
            so = np.argsort(-restc, kind="stable")
            orders[c, r0 + nb:r1] = rest[so]
        # refresh user array after reorder
        uarr[c] = np.where(orders[c] >= 0, users[np.maximum(orders[c], 0)], -1)

    # per-tile compacted seq lengths (shared across cores)
    cnt_rows = np.where(orders >= 0, cnt_all[np.maximum(orders, 0)], 0)  # [C, NPAD]
    SP = np.maximum(cnt_rows.reshape(N_CORES, NT, 128).max(-1).max(0), 1)  # [NT]
    SMAX = int(SP.max())
    roffs = np.r_[0, np.cumsum(SP * D)][:-1]
    moffs = np.r_[0, np.cumsum(SP)][:-1]
    CW = int((SP * D).sum())
    CM = int(SP.sum())
    TOTL = int(2 * sum(GS))
    loffs = np.r_[0, np.cumsum([2 * L for L in GS])][:-1]

    plan = {
        "NT": NT, "GS": GS, "SP": [int(x) for x in SP],
        "SMAX": SMAX, "CW": CW, "CM": CM, "TOTL": TOTL, "NG": NG,
        "roffs": [int(x) for x in roffs], "moffs": [int(x) for x in moffs],
        "loffs": [int(x) for x in loffs],
        "gstarts": [int(x) for x in gstarts],
    }

    wrep = _round_bf16(np.broadcast_to(w[None, :], (128, D)))

    in_maps = []
    metas = []
    for c in range(N_CORES):
        order = orders[c]
        real = order >= 0
        ridx = np.maximum(order, 0)

        # gather + compact R rows
        Rg = R_ui[ridx]                           # [NPAD, S, D]
        mg = mask[ridx] > 0
        vorder = np.argsort(~mg, axis=1, kind="stable")   # valid steps first
        Rc = np.take_along_axis(Rg, vorder[:, :, None], axis=1)
        Rc[~real] = 0.0
        cr = cnt_rows[c]                          # [NPAD]
        # zero out the tail beyond each row's count (compacted invalid steps)
        tailmask = np.arange(S)[None, :] >= cr[:, None]
        Rc[tailmask] = 0.0

        Rp2d = np.zeros((128, CW), np.float32)
        maskc = np.zeros((128, CM), np.float32)
        cntw = np.zeros((128, NT), np.float32)
        for t in range(NT):
            sp = int(SP[t])
            blk = Rc[t * 128:(t + 1) * 128, :sp, :].reshape(128, sp * D)
            Rp2d[:, roffs[t]:roffs[t] + sp * D] = blk
            ct = cr[t * 128:(t + 1) * 128]
            valid = np.arange(sp)[None, :] < ct[:, None]
            # fast path folds the *cnt scale into the mask so one fused op
            # yields wt directly; general path needs the raw 0/1 mask
            maskc[:, moffs[t]:moffs[t] + sp] = (
                valid * ct[:, None] if fast else valid)
            cntw[:, t] = ct

        q = item_emb[items[ridx]].astype(np.float32)
        q[~real] = 0.0
        qw = np.ascontiguousarray(
            q.reshape(NT, 128, E).transpose(1, 0, 2))   # [128, NT, E]

        # one-hot matrices per group
        u = uarr[c]
        ohs = np.zeros((128, TOTL, 128), np.float32)
        bnds = np.zeros((128, max(2 * (NG - 1), 1), 128), np.float32)
        prev_slot = {}
        for g in range(NG):
            r0, r1 = gstarts[g] * 128, (gstarts[g] + GS[g]) * 128
            ug = u[r0:r1]
            slot = {}
            nslot = 0
            lo = loffs[g]
            L = GS[g]
            for k in range(r1 - r0):
                uu = ug[k]
                if uu < 0:
                    continue
                j, p = divmod(k, 128)
                if g > 0 and uu in prev_slot:
                    # boundary row: scatter into previous group's bank
                    assert j == 0, "boundary row outside tile 0"
                    s0 = prev_slot[uu]
                    bnds[p, 2 * (g - 1), s0] = 1.0      # seg closer
                    bnds[s0, 2 * (g - 1) + 1, p] = 1.0  # gather
                else:
                    if uu not in slot:
                        slot[uu] = nslot
                        nslot += 1
                    s0 = slot[uu]
                    ohs[p, lo + j, s0] = 1.0            # seg own
                    ohs[s0, lo + L + j, p] = 1.0        # gather own
            assert nslot <= 128, f"slot overflow {nslot}"
            prev_slot = slot

        in_maps.append({
            "Rp2d": _round_bf16(Rp2d),
            "maskc": _round_bf16(maskc),
            "cntw": cntw,
            "qw": _round_bf16(qw),
            "wrep": wrep,
            "ohs": _round_bf16(ohs),
            "bnds": _round_bf16(bnds),
        })
        metas.append(order)
    return in_maps, metas, plan


# ----------------------------------------------------------------------------
# device program (fast path)
# ----------------------------------------------------------------------------

def build_program(plan, alpha=1.0, beta=1.0, gamma=1.0):
    nc = bacc.Bacc(
        "TRN2", target_bir_lowering=False, debug=False, num_devices=N_CORES
    )
    fast = (alpha == 1.0) and (beta == 1.0) and (gamma == 1.0)
    AF = mybir.ActivationFunctionType
    NT, GS, SP = plan["NT"], plan["GS"], plan["SP"]
    SMAX, CW, CM, TOTL, NG = (plan["SMAX"], plan["CW"], plan["CM"],
                              plan["TOTL"], plan["NG"])
    roffs, moffs, loffs = plan["roffs"], plan["moffs"], plan["loffs"]
    gstarts = plan["gstarts"]

    Rp2d = nc.dram_tensor("Rp2d", [128, CW], BF16, kind="ExternalInput")
    maskc_d = nc.dram_tensor("maskc", [128, CM], BF16, kind="ExternalInput")
    cntw_d = nc.dram_tensor("cntw", [128, NT], F32, kind="ExternalInput")
    qw_d = nc.dram_tensor("qw", [128, NT, E], BF16, kind="ExternalInput")
    wrep_d = nc.dram_tensor("wrep", [128, D], BF16, kind="ExternalInput")
    ohs_d = nc.dram_tensor("ohs", [128, TOTL, 128], BF16, kind="ExternalInput")
    bnds_d = nc.dram_tensor(
        "bnds", [128, max(2 * (NG - 1), 1), 128], BF16, kind="ExternalInput")
    r_out = nc.dram_tensor("r_out", [128, NT], F32, kind="ExternalOutput")

    add = mybir.AluOpType.add
    mult = mybir.AluOpType.mult
    mx = mybir.AluOpType.max

    with tile.TileContext(nc) as tc:
        with (
            tc.tile_pool(name="const", bufs=1) as constp,
            tc.tile_pool(name="rpool", bufs=4) as rpool,
            tc.tile_pool(name="ypool", bufs=3) as ypool,
            tc.tile_pool(name="fpool", bufs=3) as fpool,
            tc.tile_pool(name="zpool", bufs=3) as zpool,
            tc.tile_pool(name="qpool", bufs=4) as qpool,
            tc.tile_pool(name="ohpool", bufs=3) as ohpool,
            tc.tile_pool(name="bndpool", bufs=3) as bndpool,
            tc.tile_pool(name="xpool", bufs=4) as xpool,
            tc.tile_pool(name="scpool", bufs=2) as scpool,
            tc.tile_pool(name="small", bufs=4) as small,
            tc.tile_pool(name="bankp", bufs=1) as bankp,
            tc.tile_pool(name="pseg", bufs=2, space="PSUM") as pseg,
            tc.tile_pool(name="pgat", bufs=3, space="PSUM") as pgat,
        ):
            wrep_sb = constp.tile([128, 1, D], BF16)
            nc.sync.dma_start(wrep_sb[:, 0, :], wrep_d[:, :])
            if not fast:
                maskc_sb = constp.tile([128, CM], BF16)
                nc.sync.dma_start(maskc_sb[:], maskc_d[:, :])
            bank_sb = bankp.tile([128, NG, E + 1], BF16)
            wt_sb = constp.tile([128, NT], F32)
            den_sb = constp.tile([128, NT], F32)
            r_sb = constp.tile([128, NT], F32)
            cnt_sb = constp.tile([128, NT], F32)
            nc.sync.dma_start(cnt_sb[:], cntw_d[:, :])
            if fast:
                # c2 = tau * cnt^2 (bias for the wt combine)
                c2_sb = constp.tile([128, NT], F32)
                nc.vector.scalar_tensor_tensor(
                    c2_sb[:], cnt_sb[:], TAU, cnt_sb[:],
                    op0=mult, op1=mult)
                negtau = constp.tile([128, 1], F32)
                nc.vector.memset(negtau[:], -TAU)

            q_tiles = [None] * NG
            oh_tiles = [None] * NG
            bnd_tiles = [None] * NG
            ps_prev = None

            def emit_gather(g, j_lo=0, j_hi=None):
                t0, L = gstarts[g], GS[g]
                if j_hi is None:
                    j_hi = L
                ohg = oh_tiles[g]
                qg = q_tiles[g]
                for j in range(j_lo, min(j_hi, L)):
                    t = t0 + j
                    gp = pgat.tile([128, E + 1], F32, name="gp")
                    if g > 0 and j == 0:
                        nc.tensor.matmul(
                            gp[:], bnd_tiles[g][:, 1, :], bank_sb[:, g - 1, :],
                            start=True, stop=False)
                        nc.tensor.matmul(
                            gp[:], ohg[:, L + j, :], bank_sb[:, g, :],
                            start=False, stop=True)
                    else:
                        nc.tensor.matmul(
                            gp[:], ohg[:, L + j, :], bank_sb[:, g, :],
                            start=True, stop=True)
                    nc.scalar.copy(den_sb[:, t:t + 1], gp[:, E:E + 1])
                    # ACT drains PSUM as bf16 so the r_num stt runs in DVE
                    # 2x mode on SBUF (tensor_tensor_reduce is a direct-ISA
                    # op that the PJRT execution path cannot run; stt is
                    # the equivalent fused mul+accum)
                    nb = scpool.tile([128, E], BF16, tag="nb")
                    nc.scalar.copy(nb[:], gp[:, 0:E])
                    sc = scpool.tile([128, E], BF16, tag="sc")
                    nc.vector.scalar_tensor_tensor(
                        sc[:], nb[:], 1.0, qg[:, j, :],
                        op0=mult, op1=mult, accum_out=r_sb[:, t:t + 1])

            # ---- phase A: Z -> wt -> X -> segment matmuls into group banks
            #
            # The Z pipeline runs over PAIRS of tiles (one op per pair,
            # halving per-op overhead; R columns of adjacent tiles are
            # contiguous in Rp2d) and is SOFTWARE-PIPELINED: pair k+1's
            # multiply is emitted before pair k's fold tail so the in-order
            # DVE queue never blocks on the GPSIMD fold1 of pair k.
            pairs = []
            for g in range(NG):
                L = GS[g]
                for j0 in range(0, L, 2):
                    pairs.append((g, j0, min(2, L - j0)))

            ps_state = {"ps": None, "ps_prev": None}

            def stage_a(pair):
                g, j0, npair = pair
                t0, L = gstarts[g], GS[g]
                if j0 == 0:
                    ohg = ohpool.tile([128, 2 * LCAP, 128], BF16, name="ohg")
                    nc.scalar.dma_start(
                        ohg[:, 0:2 * L, :],
                        ohs_d[:, loffs[g]:loffs[g] + 2 * L, :])
                    oh_tiles[g] = ohg
                    if g > 0:
                        bnd = bndpool.tile([128, 2, 128], BF16, name="bnd")
                        nc.scalar.dma_start(
                            bnd[:], bnds_d[:, 2 * (g - 1):2 * g, :])
                        bnd_tiles[g] = bnd
                    qg = qpool.tile([128, LCAP, E], BF16, name="qg")
                    nc.scalar.dma_start(
                        qg[:, 0:L, :], qw_d[:, t0:t0 + L, :])
                    q_tiles[g] = qg
                sps = [SP[t0 + j0 + k] for k in range(npair)]
                tot = sum(sps)
                rt = rpool.tile([128, 2 * SMAX * D], BF16, name="rt")
                nc.sync.dma_start(
                    rt[:, 0:tot * D],
                    Rp2d[:, roffs[t0 + j0]:roffs[t0 + j0] + tot * D])
                rv = rt[:, 0:tot * D].rearrange("p (s d) -> p s d", d=D)
                y = ypool.tile([128, 2 * SMAX, D], BF16, name="y")
                nc.vector.tensor_mul(
                    y[:, 0:tot, :], rv,
                    wrep_sb[:].broadcast_to((128, tot, D)))
                # fold1 mostly on the (otherwise idle) GPSIMD engine; a small
                # slice stays on DVE to balance the two engines' busy time
                f1 = fpool.tile([128, 2 * SMAX, 32], BF16, tag="f1", name="f1")
                sa = max(1, int(tot * 0.95))
                nc.gpsimd.tensor_add(
                    f1[:, 0:sa, :], y[:, 0:sa, 0:32], y[:, 0:sa, 32:64])
                if sa < tot:
                    nc.vector.tensor_add(
                        f1[:, sa:tot, :], y[:, sa:tot, 0:32],
                        y[:, sa:tot, 32:64])
                return (pair, sps, tot, f1)

            def stage_b(st):
                (g, j0, npair), sps, tot, f1 = st
                t0, L = gstarts[g], GS[g]
                ohg = oh_tiles[g]
                qg = q_tiles[g]
                f2 = fpool.tile([128, 2 * SMAX, 16], BF16, tag="f2", name="f2")
                nc.vector.tensor_add(
                    f2[:, 0:tot, :], f1[:, 0:tot, 0:16], f1[:, 0:tot, 16:32])
                f3 = fpool.tile([128, 2 * SMAX, 8], BF16, tag="f3", name="f3")
                nc.vector.tensor_add(
                    f3[:, 0:tot, :], f2[:, 0:tot, 0:8], f2[:, 0:tot, 8:16])
                f4 = fpool.tile([128, 2 * SMAX, 4], BF16, tag="f4", name="f4")
                nc.vector.tensor_add(
                    f4[:, 0:tot, :], f3[:, 0:tot, 0:4], f3[:, 0:tot, 4:8])
                z = zpool.tile([128, 2 * SMAX], F32, tag="z", name="z")
                nc.vector.tensor_reduce(
                    z[:, 0:tot], f4[:, 0:tot, :],
                    axis=mybir.AxisListType.X, op=add)

                if j0 == 0:
                    ps_state["ps_prev"] = ps_state["ps"]
                    ps_state["ps"] = pseg.tile([128, E + 1], F32, name="ps")
                ps = ps_state["ps"]
                ps_prev = ps_state["ps_prev"]

                off = 0
                for k in range(npair):
                    j = j0 + k
                    t = t0 + j
                    sp = sps[k]
                    zt = z[:, off:off + sp]
                    off += sp
                    wt_col = wt_sb[:, t:t + 1]
                    if fast:
                        # compacted tail is exactly 0 so relu(z-tau) drops
                        # it: wt = cnt*A + tau*cnt^2 with
                        # A = sum_s relu(z - tau). Runs on ACT only.
                        za = zpool.tile([128, SMAX], F32, tag="za", name="za")
                        a_col = small.tile([128, 1], F32, tag="a", name="ac")
                        nc.scalar.activation(
                            za[:, 0:sp], zt, AF.Relu, bias=negtau[:],
                            accum_out=a_col)
                        nc.scalar.activation(
                            wt_col, a_col[:], AF.Relu,
                            bias=c2_sb[:, t:t + 1], scale=cnt_sb[:, t:t + 1])
                    else:
                        wp = zpool.tile([128, SMAX], F32, tag="wp", name="wp")
                        # A = sum_s mask * max(z,tau)^alpha; wt =
                        # exp((gamma/alpha)*(ln A + alpha*beta*ln cnt))
                        za = zpool.tile([128, SMAX], F32, tag="za", name="za")
                        nc.vector.tensor_scalar_max(za[:, 0:sp], zt, TAU)
                        nc.scalar.activation(za[:, 0:sp], za[:, 0:sp], AF.Log)
                        nc.scalar.activation(
                            za[:, 0:sp], za[:, 0:sp], AF.Exp,
                            scale=float(alpha))
                        a_col = small.tile([128, 1], F32, tag="a", name="ac")
                        nc.vector.scalar_tensor_tensor(
                            wp[:, 0:sp], za[:, 0:sp], 0.0,
                            maskc_sb[:, moffs[t]:moffs[t] + sp],
                            op0=add, op1=mult, accum_out=a_col)
                        la = small.tile([128, 1], F32, tag="la", name="la")
                        nc.scalar.activation(la[:], a_col[:], AF.Log)
                        lc = small.tile([128, 1], F32, tag="lc", name="lc")
                        nc.scalar.activation(lc[:], cnt_sb[:, t:t + 1], AF.Log)
                        nc.vector.scalar_tensor_tensor(
                            la[:], lc[:], float(alpha * beta), la[:],
                            op0=mult, op1=add)
                        nc.scalar.activation(
                            wt_col, la[:], AF.Exp, scale=float(gamma / alpha))

                    xt = xpool.tile([128, E + 1], BF16, name="xt")
                    nc.scalar.mul(xt[:, 0:E], qg[:, j, :], wt_col)
                    nc.scalar.copy(xt[:, E:E + 1], wt_col)

                    last = (g == NG - 1) and (j == L - 1)
                    nc.tensor.matmul(
                        ps[:], ohg[:, j, :], xt[:],
                        start=(j == 0), stop=last)
                    if j == 0 and g > 0:
                        nc.tensor.matmul(
                            ps_prev[:], bnd_tiles[g][:, 0, :], xt[:],
                            start=False, stop=True)
                        nc.scalar.copy(bank_sb[:, g - 1, :], ps_prev[:])
                    if last:
                        nc.scalar.copy(bank_sb[:, g, :], ps[:])
                # bank g-1 closes at this group's first pair; spread its
                # gather tiles across this group's pairs so the DVE stream
                # stays uniform instead of bursting at group boundaries
                if g > 0:
                    if j0 + npair >= L:
                        emit_gather(g - 1, j0, GS[g - 1])
                    else:
                        emit_gather(g - 1, j0, j0 + npair)

            prev = None
            for pair in pairs:
                st = stage_a(pair)
                if prev is not None:
                    stage_b(prev)
                prev = st
            stage_b(prev)
            emit_gather(NG - 1)

            # pad lanes have den=0 (all-zero one-hot rows); clamp so the
            # reciprocal stays finite and r stays 0 instead of NaN
            rec = constp.tile([128, NT], F32)
            nc.vector.tensor_scalar_max(den_sb[:], den_sb[:], 1e-30)
            nc.vector.reciprocal(rec[:], den_sb[:])
            nc.vector.tensor_mul(r_sb[:], r_sb[:], rec[:])
            nc.sync.dma_start(r_out[:, :], r_sb[:])

    nc.compile()
    return nc


# ----------------------------------------------------------------------------
# entry point
# ----------------------------------------------------------------------------

def kernel(users, items, R_ui, mask, w, item_emb, alpha, beta, gamma,
           _return_extras=False, _trace=False):
    users = np.asarray(users, np.int64)
    items = np.asarray(items, np.int64)
    R_ui = np.asarray(R_ui, np.float32)
    mask_f = np.asarray(mask).astype(np.float32)
    w = np.asarray(w, np.float32)
    item_emb = np.asarray(item_emb, np.float32)
    al = float(np.asarray(alpha).reshape(-1)[0])
    be = float(np.asarray(beta).reshape(-1)[0])
    ga = float(np.asarray(gamma).reshape(-1)[0])

    fast = (al == 1.0) and (be == 1.0) and (ga == 1.0)

    import time as _time

    t0 = _time.perf_counter()
    in_maps, metas, plan = _preprocess(
        users, items, R_ui, mask_f, w, item_emb, fast=fast)
    t1 = _time.perf_counter()
    nc = build_program(plan, al, be, ga)
    t2 = _time.perf_counter()
    res = run_bass_kernel_spmd(
        nc, in_maps, core_ids=list(range(N_CORES)), trace=_trace
    )
    t3 = _time.perf_counter()
    print(
        f"[kernel] preprocess {t1-t0:.1f}s  build+schedule {t2-t1:.1f}s  "
        f"compile+run {t3-t2:.1f}s"
    )

    n = users.shape[0]
    r = np.empty(n, np.float32)
    for c in range(N_CORES):
        order = metas[c]
        flat = np.asarray(res.results[c]["r_out"], np.float32).T.reshape(-1)
        valid = order >= 0
        r[order[valid]] = flat[valid]
    if _return_extras:
        return r, res
    return r
